# revision 16
# baseline (speedup 1.0000x reference)
"""Bahdanau attention with coverage — Trainium2 Bass kernel, 8-core data-parallel.

Math (per batch b):
  enc_p = X_b @ Wh + bh            X_b: (S=2048, H=1024)
  dec_p = d_b @ Ws + bs            (H,)
  cov_p = c_b @ Wc + bc            scalar
  score = tanh(enc_p + dec_p + cov_p) @ V + bv      (bv dropped: softmax shift-invariant)
  attn  = softmax(score)           over S
  cov_new = c_b + attn
  ctx   = attn^T @ X_b             (H,)

Strategy: batch-sharded over 8 cores (4 batches each). Compute enc_p in the
transposed layout enc_p^T (H on partitions, S on free) so the bias add + tanh
fuse into one ScalarE activation and the V-dot contracts over partitions.
The host stages X in both layouts (transposed for the enc_p matmul, natural
streamed just-in-time for the context matmul) so no on-chip transposes of X
are needed. All big matmuls run in fp32r (1 cycle/row at N=512, ~16x more
accurate than bf16). Wh is loaded in k-major 128x128 blocks so the first
matmul group only waits for 512KB of weights.

Sequence index mapping is plain s-major: column c of U <-> s = c; attn tile
(p, m) <-> s = 128 m + p. attn/coverage get one small PE transpose to a
(16, 128) layout so the output DMAs are 512B-contiguous.
"""

import numpy as np

B, S, H = 32, 2048, 1024
NCORES = 8
BL = B // NCORES          # batches per core
NK = H // 128             # 8 h/k tiles
NJ = 16                   # 128-row s-tiles per batch
NG = 4                    # s-blocks of 512 per batch
SBW = 512                 # s-block width

_CACHE = {}


def _build():
    import concourse.bacc as bacc
    import concourse.mybir as mybir
    import concourse.tile as tile
    from concourse.masks import make_identity

    f32 = mybir.dt.float32
    f32r = mybir.dt.float32r
    AF = mybir.ActivationFunctionType
    ALU = mybir.AluOpType

    nc = bacc.Bacc("TRN2", target_bir_lowering=False, debug=False,
                   num_devices=NCORES)

    eo = nc.dram_tensor("eo", (BL, S, H), f32, kind="ExternalInput").ap()
    eo_t = nc.dram_tensor("eo_t", (BL, H, S), f32, kind="ExternalInput").ap()
    dec = nc.dram_tensor("dec", (BL, H), f32, kind="ExternalInput").ap()
    cov = nc.dram_tensor("cov", (BL, S), f32, kind="ExternalInput").ap()
    wh_d = nc.dram_tensor("wh", (H, H), f32, kind="ExternalInput").ap()
    bh_d = nc.dram_tensor("bh", (H,), f32, kind="ExternalInput").ap()
    ws_d = nc.dram_tensor("ws", (H, H), f32, kind="ExternalInput").ap()
    bs_d = nc.dram_tensor("bs", (H,), f32, kind="ExternalInput").ap()
    v_d = nc.dram_tensor("v", (H, 1), f32, kind="ExternalInput").ap()
    wc_d = nc.dram_tensor("wc", (S, 1), f32, kind="ExternalInput").ap()
    bc_d = nc.dram_tensor("bc", (1,), f32, kind="ExternalInput").ap()

    ctx_o = nc.dram_tensor("ctx_o", (BL, H), f32, kind="ExternalOutput").ap()
    attn_o = nc.dram_tensor("attn_o", (BL, S, 1), f32, kind="ExternalOutput").ap()
    covn_o = nc.dram_tensor("covn_o", (BL, S), f32, kind="ExternalOutput").ap()

    with tile.TileContext(nc) as tc:
        with (
            tc.tile_pool(name="const", bufs=1) as cp,
            tc.tile_pool(name="wraw", bufs=2) as wraw,
            tc.tile_pool(name="wst", bufs=4) as wstp,
            tc.tile_pool(name="whr", bufs=1) as whrp,
            tc.tile_pool(name="xtr", bufs=4) as xtrp,
            tc.tile_pool(name="xt", bufs=2) as xtp,
            tc.tile_pool(name="xc32", bufs=3) as xc32p,
            tc.tile_pool(name="xcr", bufs=12) as xcrp,
            tc.tile_pool(name="tt", bufs=3) as tp,
            tc.tile_pool(name="u", bufs=2) as up,
            tc.tile_pool(name="sm", bufs=2) as smp,
            tc.tile_pool(name="ps_e", bufs=3, space="PSUM") as ps_e,
            tc.tile_pool(name="ps_s", bufs=2, space="PSUM") as ps_s,
            tc.tile_pool(name="ps_c", bufs=2, space="PSUM") as ps_c,
        ):
            # ------- small input DMAs first (sync queue)
            v8 = cp.tile([8, 128], f32)
            nc.sync.dma_start(v8[:], v_d.rearrange("(t p) one -> t (p one)", t=8))
            bh8 = cp.tile([8, 128], f32)
            nc.sync.dma_start(bh8[:], bh_d.rearrange("(t p) -> t p", t=8))
            bs8 = cp.tile([8, 128], f32)
            nc.sync.dma_start(bs8[:], bs_d.rearrange("(t p) -> t p", t=8))
            d8 = wraw.tile([BL, H], f32, tag="wraw")
            nc.sync.dma_start(d8[:], dec[:, :])
            wc_sb = cp.tile([NJ, 128], f32)
            nc.sync.dma_start(wc_sb[:], wc_d.rearrange("(m q) one -> m (q one)", q=128))
            bc_sb = cp.tile([1, 1], f32)
            nc.sync.dma_start(bc_sb[:], bc_d.rearrange("(one two) -> one two", one=1))
            c_sbs = []
            for b in range(BL):
                c_sb = smp.tile([NJ, 128], f32, tag="c", bufs=BL)
                nc.sync.dma_start(c_sb[:], cov[b].rearrange("(m q) -> m q", q=128))
                c_sbs.append(c_sb)

            # ---------------- constants ----------------
            ident_f = cp.tile([128, 128], f32)
            make_identity(nc, ident_f[:])
            ident_r = cp.tile([128, 128], f32r)
            nc.vector.tensor_copy(ident_r[:], ident_f[:])

            ones = cp.tile([128, 1], f32)
            nc.vector.memset(ones[:], 1.0)
            ones1 = cp.tile([1, 128], f32)
            nc.vector.memset(ones1[:], 1.0)

            # Wh in k-major 128x128 blocks, rounded to fp32r (scalar queue):
            # wh_r[:, (k*8+t)*128 + c] = Wh[128 t + p, 128 k + c]
            # k=0 blocks load first, then the first s-block of batch 0 is
            # prefetched on the same queue, then the remaining k blocks.
            wh_r = whrp.tile([128, NK * H], f32r)

            def load_wh_k(k):
                for t in range(NK):
                    wb = wstp.tile([128, 128], f32, tag="wst")
                    nc.sync.dma_start(
                        wb[:], wh_d[t * 128:(t + 1) * 128, k * 128:(k + 1) * 128])
                    nc.vector.tensor_copy(
                        wh_r[:, (k * NK + t) * 128:(k * NK + t + 1) * 128], wb[:])

            def load_xt(b, g, split=False):
                xt = xtp.tile([128, NK * SBW], f32r, tag="xt", name=f"xt_{b}_{g}")
                for t in range(NK):
                    xtr = xtrp.tile([128, SBW], f32, tag="xtr")
                    eng = nc.scalar if (split and t % 2) else nc.sync
                    eng.dma_start(
                        xtr[:], eo_t[b, t * 128:(t + 1) * 128,
                                     g * SBW:(g + 1) * SBW])
                    nc.vector.tensor_copy(xt[:, t * SBW:(t + 1) * SBW], xtr[:])
                return xt

            load_wh_k(0)
            xt_prefetch = {(0, 0): load_xt(0, 0)}
            for k in range(1, NK):
                load_wh_k(k)

            # V -> v_sb[p, t] = V[128 t + p]
            v8r = cp.tile([8, 128], f32r)
            nc.vector.tensor_copy(v8r[:], v8[:])
            ps_v = ps_s.tile([128, 8], f32r, tag="small")
            nc.tensor.transpose(ps_v[:], v8r[:], ident_r[0:8, 0:8])
            v_sb = cp.tile([128, 8], f32)
            nc.vector.tensor_copy(v_sb[:], ps_v[:])

            # (bh + bs) -> bhsT[p, t]
            bhs8 = cp.tile([8, 128], f32r)
            nc.vector.tensor_add(out=bhs8[:], in0=bh8[:], in1=bs8[:])
            ps_b = ps_s.tile([128, 8], f32r, tag="small")
            nc.tensor.transpose(ps_b[:], bhs8[:], ident_r[0:8, 0:8])
            bhsT = cp.tile([128, 8], f32)
            nc.vector.tensor_copy(bhsT[:], ps_b[:])

            # decoder state -> dT[p, 4t + b] = d[b, 128 t + p]  (fp32r)
            d8r = cp.tile([BL, H], f32r)
            nc.vector.tensor_copy(d8r[:], d8[:])
            dT = cp.tile([128, NK * BL], f32r)
            for t in range(NK):
                ps_d = ps_s.tile([128, BL], f32r, tag="small")
                nc.tensor.transpose(ps_d[:], d8r[0:BL, t * 128:(t + 1) * 128],
                                    ident_r[0:BL, 0:BL])
                nc.vector.tensor_copy(dT[:, t * BL:(t + 1) * BL], ps_d[:])

            # cov_p per batch (c, wc in (16,128) layout)
            cc = cp.tile([NJ, BL], f32)
            junk = cp.tile([NJ, 128], f32)
            for b in range(BL):
                nc.vector.tensor_mul(out=junk[:], in0=c_sbs[b][:], in1=wc_sb[:])
                nc.vector.reduce_sum(out=cc[0:NJ, b:b + 1], in_=junk[:],
                                     axis=mybir.AxisListType.X)
            ps_cp = ps_s.tile([1, BL], f32, tag="small")
            nc.tensor.matmul(ps_cp[:], ones[0:NJ, :], cc[:], start=True, stop=True)
            covp1 = cp.tile([1, BL], f32)
            nc.vector.tensor_scalar(out=covp1[:], in0=ps_cp[:], scalar1=bc_sb[:],
                                    scalar2=None, op0=ALU.add)
            ps_cb = ps_s.tile([128, BL], f32, tag="small")
            nc.tensor.matmul(ps_cb[:], ones1[:], covp1[:], start=True, stop=True)
            covp_bc = cp.tile([128, BL], f32)
            nc.vector.tensor_copy(covp_bc[:], ps_cb[:])

            # dec_p in natural layout: (4 b, 1024 k) = dT.T @ Ws, fp32r N=512.
            # Ws streams on the gpsimd (SWDGE) queue.
            ps_dn0 = ps_c.tile([BL, SBW], f32, tag="ctx")
            ps_dn1 = ps_s.tile([BL, SBW], f32, tag="small")
            for t in range(NK):
                wst = wraw.tile([128, H], f32, tag="wraw")
                nc.scalar.dma_start(wst[:], ws_d[t * 128:(t + 1) * 128, :])
                wsr = wraw.tile([128, H], f32r, tag="wsr")
                nc.vector.tensor_copy(wsr[:], wst[:])
                nc.tensor.matmul(ps_dn0[:], dT[:, t * BL:(t + 1) * BL],
                                 wsr[:, 0:SBW], start=(t == 0), stop=(t == NK - 1))
                nc.tensor.matmul(ps_dn1[:], dT[:, t * BL:(t + 1) * BL],
                                 wsr[:, SBW:H], start=(t == 0), stop=(t == NK - 1))
            dn_sb = cp.tile([BL, H], f32r)
            nc.vector.tensor_copy(dn_sb[0:BL, 0:SBW], ps_dn0[:])
            nc.vector.tensor_copy(dn_sb[0:BL, SBW:H], ps_dn1[:])

            # bias_sb[p, 4 k + b] = dec_p^T + (bh+bs)^T + cov_p
            bias_sb = cp.tile([128, NK * BL], f32)
            for k in range(NK):
                ps_dk = ps_s.tile([128, BL], f32r, tag="small")
                nc.tensor.transpose(ps_dk[:], dn_sb[0:BL, k * 128:(k + 1) * 128],
                                    ident_r[0:BL, 0:BL])
                nc.vector.tensor_scalar(
                    out=bias_sb[:, k * BL:(k + 1) * BL], in0=ps_dk[:],
                    scalar1=bhsT[:, k:k + 1], scalar2=None, op0=ALU.add)
                nc.vector.tensor_add(
                    out=bias_sb[:, k * BL:(k + 1) * BL],
                    in0=bias_sb[:, k * BL:(k + 1) * BL], in1=covp_bc[:])

            # ---------------- main per-batch pipeline ----------------
            # ctx matmuls of batch b are emitted after batch b+1's first
            # dense main-matmul block so the PE stays HAM-warm across the
            # batch seam and the streamed xcr tiles have time to arrive.
            ctx_stash = [None]

            for b in range(BL):
                U = up.tile([128, S], f32, tag="u")

                for g in range(NG):
                    xt = xt_prefetch.pop((b, g), None)
                    if xt is None:
                        xt = load_xt(b, g)

                    if g == 1 and ctx_stash[0] is not None:
                        ctx_stash[0]()
                        ctx_stash[0] = None

                    for k in range(NK):
                        ps_mm = ps_e.tile([128, SBW], f32, tag="mm")
                        for t in range(NK):
                            nc.tensor.matmul(
                                ps_mm[:],
                                wh_r[:, (k * NK + t) * 128:(k * NK + t + 1) * 128],
                                xt[:, t * SBW:(t + 1) * SBW],
                                start=(t == 0), stop=(t == NK - 1))
                        tt = tp.tile([128, SBW], f32, tag="t")
                        nc.scalar.activation(tt[:], ps_mm[:], AF.Tanh,
                                             bias=bias_sb[:, k * BL + b: k * BL + b + 1],
                                             scale=1.0)
                        if k == 0:
                            nc.vector.tensor_scalar(
                                out=U[:, g * SBW:(g + 1) * SBW], in0=tt[:],
                                scalar1=v_sb[:, k:k + 1], scalar2=None, op0=ALU.mult)
                        else:
                            nc.vector.affine_then_add(
                                out=U[:, g * SBW:(g + 1) * SBW], in0=tt[:],
                                in1=U[:, g * SBW:(g + 1) * SBW],
                                scale=v_sb[:, k:k + 1], bias=0.0)

                # natural X m-tiles for context, streamed just-in-time
                # (gpsimd SWDGE queue; consumed by the ctx matmuls below)
                xcs = []
                for m in range(NJ):
                    xc32 = xc32p.tile([128, H], f32, tag="xc32")
                    nc.gpsimd.dma_start(xc32[:], eo[b, m * 128:(m + 1) * 128, :])
                    xcr = xcrp.tile([128, H], f32r, tag="xcr",
                                    name=f"xcr_{b}_{m}")
                    if m % 2 == 0:
                        nc.vector.tensor_copy(xcr[:], xc32[:])
                    else:
                        nc.scalar.copy(xcr[:], xc32[:])
                    xcs.append(xcr)

                # score^T: score_sb[p, m] = score[s = 128 m + p]
                score_sb = smp.tile([128, NJ], f32, tag="score")
                for m in range(NJ):
                    ps_sc = ps_s.tile([128, 1], f32, tag="small")
                    nc.tensor.matmul(ps_sc[:], U[:, m * 128:(m + 1) * 128], ones[:],
                                     start=True, stop=True)
                    nc.scalar.copy(score_sb[:, m:m + 1], ps_sc[:])

                # softmax over all 2048 (no max-subtraction; |score| <= ~16)
                esb = smp.tile([128, NJ], f32, tag="esb")
                acc = smp.tile([128, 1], f32, tag="acc")
                nc.scalar.activation(esb[:], score_sb[:], AF.Exp, accum_out=acc[:])
                ps_tot = ps_s.tile([1, 1], f32, tag="small")
                nc.tensor.matmul(ps_tot[:], acc[:], ones[:], start=True, stop=True)
                tot1 = smp.tile([1, 1], f32, tag="tot")
                nc.scalar.copy(tot1[:], ps_tot[:])
                rcp1 = smp.tile([1, 1], f32, tag="rcp")
                nc.vector.reciprocal(rcp1[:], tot1[:])
                ps_rb = ps_s.tile([128, 1], f32, tag="small")
                nc.tensor.matmul(ps_rb[:], ones1[:], rcp1[:], start=True, stop=True)
                rcp_sb = smp.tile([128, 1], f32, tag="rcpb")
                nc.scalar.copy(rcp_sb[:], ps_rb[:])

                attn_f = smp.tile([128, NJ], f32, tag="attnf")
                nc.vector.tensor_scalar_mul(attn_f[:], esb[:], rcp_sb[:])
                attn_r = smp.tile([128, NJ], f32r, tag="attnr")
                nc.vector.tensor_scalar_mul(attn_r[:], esb[:], rcp_sb[:])

                ps_at = ps_s.tile([NJ, 128], f32, tag="small")
                nc.tensor.transpose(ps_at[:], attn_f[:], ident_f[:])
                attn16 = smp.tile([NJ, 128], f32, tag="attn16")
                nc.vector.tensor_copy(attn16[:], ps_at[:])
                covn16 = smp.tile([NJ, 128], f32, tag="covn16")
                nc.vector.tensor_add(out=covn16[:], in0=c_sbs[b][:], in1=attn16[:])

                nc.gpsimd.dma_start(
                    attn_o[b].rearrange("(m q) one -> m (q one)", q=128), attn16[:])
                nc.gpsimd.dma_start(
                    covn_o[b].rearrange("(m q) -> m q", q=128), covn16[:])

                # context: ctx[h] = sum_m sum_p attn[p, m] * X[128 m + p, h]
                # m-major so each streamed xcr tile's two reads are adjacent
                def emit_ctx(b=b, attn_r=attn_r, xcs=xcs):
                    ctx_sb = smp.tile([1, H], f32, tag="ctxsb",
                                      name=f"ctx_sb_{b}")
                    ps_cx0 = ps_c.tile([1, SBW], f32, tag="ctx",
                                       name=f"ps_cx0_{b}")
                    ps_cx1 = ps_c.tile([1, SBW], f32, tag="ctx",
                                       name=f"ps_cx1_{b}")
                    for m in range(NJ):
                        nc.tensor.matmul(
                            ps_cx0[:], attn_r[:, m:m + 1], xcs[m][:, 0:SBW],
                            start=(m == 0), stop=(m == NJ - 1))
                        nc.tensor.matmul(
                            ps_cx1[:], attn_r[:, m:m + 1], xcs[m][:, SBW:H],
                            start=(m == 0), stop=(m == NJ - 1))
                    nc.scalar.copy(ctx_sb[:, 0:SBW], ps_cx0[:])
                    nc.scalar.copy(ctx_sb[:, SBW:H], ps_cx1[:])
                    nc.gpsimd.dma_start(ctx_o[b:b + 1, :], ctx_sb[:])

                if b < BL - 1:
                    ctx_stash[0] = emit_ctx
                else:
                    emit_ctx()

    nc.compile()
    return nc


def _get_nc():
    if "nc" not in _CACHE:
        _CACHE["nc"] = _build()
    return _CACHE["nc"]


def run(trace=False, **inputs):
    from concourse.bass_utils import run_bass_kernel_spmd

    nc = _get_nc()
    f32 = np.float32

    def c(x):
        return np.ascontiguousarray(np.asarray(x, dtype=f32))

    in_maps = []
    for i in range(NCORES):
        sl = slice(i * BL, (i + 1) * BL)
        eo_sl = c(inputs["encoder_output"][sl])
        in_maps.append({
            "eo": eo_sl,
            "eo_t": np.ascontiguousarray(eo_sl.transpose(0, 2, 1)),
            "dec": c(inputs["decoder_state"][sl]),
            "cov": c(inputs["coverage_vector"][sl]),
            "wh": c(inputs["Wh"]),
            "bh": c(inputs["bh"]),
            "ws": c(inputs["Ws"]),
            "bs": c(inputs["bs"]),
            "v": c(inputs["V"]),
            "wc": c(inputs["Wc"]),
            "bc": c(inputs["bc"]),
        })

    res = run_bass_kernel_spmd(nc, in_maps, list(range(NCORES)), trace=trace)

    ctx = np.concatenate([res.results[i]["ctx_o"] for i in range(NCORES)], axis=0)
    attn = np.concatenate([res.results[i]["attn_o"] for i in range(NCORES)], axis=0)
    covn = np.concatenate([res.results[i]["covn_o"] for i in range(NCORES)], axis=0)
    return (ctx, attn, covn), res


def kernel(**inputs):
    outs, _ = run(trace=False, **inputs)
    return outs


# revision 17
# speedup vs baseline: 1.1043x; 1.1043x over previous
"""Bahdanau attention with coverage — Trainium2 Bass kernel, 8-core data-parallel.

Math (per batch b):
  enc_p = X_b @ Wh + bh            X_b: (S=2048, H=1024)
  dec_p = d_b @ Ws + bs            (H,)
  cov_p = c_b @ Wc + bc            scalar
  score = tanh(enc_p + dec_p + cov_p) @ V + bv      (bv dropped: softmax shift-invariant)
  attn  = softmax(score)           over S
  cov_new = c_b + attn
  ctx   = attn^T @ X_b             (H,)

Strategy: batch-sharded over 8 cores (4 batches each). Compute enc_p in the
transposed layout enc_p^T (H on partitions, S on free) so the bias add + tanh
fuse into one ScalarE activation and the V-dot contracts over partitions.
The host stages X in both layouts (transposed for the enc_p matmul, natural
streamed just-in-time for the context matmul) so no on-chip transposes of X
are needed. All big matmuls run in fp32r (1 cycle/row at N=512, ~16x more
accurate than bf16). Wh is loaded in k-major 128x128 blocks so the first
matmul group only waits for 512KB of weights.

Sequence index mapping is plain s-major: column c of U <-> s = c; attn tile
(p, m) <-> s = 128 m + p. attn/coverage get one small PE transpose to a
(16, 128) layout so the output DMAs are 512B-contiguous.
"""

import numpy as np

B, S, H = 32, 2048, 1024
NCORES = 8
BL = B // NCORES          # batches per core
NK = H // 128             # 8 h/k tiles
NJ = 16                   # 128-row s-tiles per batch
NG = 4                    # s-blocks of 512 per batch
SBW = 512                 # s-block width

_CACHE = {}


def _build():
    import concourse.bacc as bacc
    import concourse.mybir as mybir
    import concourse.tile as tile
    from concourse.masks import make_identity

    f32 = mybir.dt.float32
    f32r = mybir.dt.float32r
    AF = mybir.ActivationFunctionType
    ALU = mybir.AluOpType

    nc = bacc.Bacc("TRN2", target_bir_lowering=False, debug=False,
                   num_devices=NCORES)

    eo = nc.dram_tensor("eo", (BL, S, H), f32, kind="ExternalInput").ap()
    eo_t = nc.dram_tensor("eo_t", (BL, H, S), f32, kind="ExternalInput").ap()
    dec = nc.dram_tensor("dec", (BL, H), f32, kind="ExternalInput").ap()
    cov = nc.dram_tensor("cov", (BL, S), f32, kind="ExternalInput").ap()
    wh_d = nc.dram_tensor("wh", (H, H), f32, kind="ExternalInput").ap()
    bh_d = nc.dram_tensor("bh", (H,), f32, kind="ExternalInput").ap()
    ws_d = nc.dram_tensor("ws", (H, H), f32, kind="ExternalInput").ap()
    bs_d = nc.dram_tensor("bs", (H,), f32, kind="ExternalInput").ap()
    v_d = nc.dram_tensor("v", (H, 1), f32, kind="ExternalInput").ap()
    wc_d = nc.dram_tensor("wc", (S, 1), f32, kind="ExternalInput").ap()
    bc_d = nc.dram_tensor("bc", (1,), f32, kind="ExternalInput").ap()

    ctx_o = nc.dram_tensor("ctx_o", (BL, H), f32, kind="ExternalOutput").ap()
    attn_o = nc.dram_tensor("attn_o", (BL, S, 1), f32, kind="ExternalOutput").ap()
    covn_o = nc.dram_tensor("covn_o", (BL, S), f32, kind="ExternalOutput").ap()

    with tile.TileContext(nc) as tc:
        with (
            tc.tile_pool(name="const", bufs=1) as cp,
            tc.tile_pool(name="wraw", bufs=2) as wraw,
            tc.tile_pool(name="wst", bufs=4) as wstp,
            tc.tile_pool(name="whr", bufs=1) as whrp,
            tc.tile_pool(name="xtr", bufs=4) as xtrp,
            tc.tile_pool(name="xt", bufs=2) as xtp,
            tc.tile_pool(name="xc32", bufs=3) as xc32p,
            tc.tile_pool(name="xcr", bufs=12) as xcrp,
            tc.tile_pool(name="tt", bufs=3) as tp,
            tc.tile_pool(name="u", bufs=2) as up,
            tc.tile_pool(name="sm", bufs=2) as smp,
            tc.tile_pool(name="ps_e", bufs=3, space="PSUM") as ps_e,
            tc.tile_pool(name="ps_s", bufs=2, space="PSUM") as ps_s,
            tc.tile_pool(name="ps_c", bufs=2, space="PSUM") as ps_c,
        ):
            # ------- small input DMAs first (sync queue)
            v8 = cp.tile([8, 128], f32)
            nc.sync.dma_start(v8[:], v_d.rearrange("(t p) one -> t (p one)", t=8))
            bh8 = cp.tile([8, 128], f32)
            nc.sync.dma_start(bh8[:], bh_d.rearrange("(t p) -> t p", t=8))
            bs8 = cp.tile([8, 128], f32)
            nc.sync.dma_start(bs8[:], bs_d.rearrange("(t p) -> t p", t=8))
            d8 = wraw.tile([BL, H], f32, tag="wraw")
            nc.sync.dma_start(d8[:], dec[:, :])
            wc_sb = cp.tile([NJ, 128], f32)
            nc.sync.dma_start(wc_sb[:], wc_d.rearrange("(m q) one -> m (q one)", q=128))
            bc_sb = cp.tile([1, 1], f32)
            nc.sync.dma_start(bc_sb[:], bc_d.rearrange("(one two) -> one two", one=1))
            c_sbs = []
            for b in range(BL):
                c_sb = smp.tile([NJ, 128], f32, tag="c", bufs=BL)
                nc.sync.dma_start(c_sb[:], cov[b].rearrange("(m q) -> m q", q=128))
                c_sbs.append(c_sb)

            # ---------------- constants ----------------
            ident_f = cp.tile([128, 128], f32)
            make_identity(nc, ident_f[:])
            ident_r = cp.tile([128, 128], f32r)
            nc.vector.tensor_copy(ident_r[:], ident_f[:])

            ones = cp.tile([128, 1], f32)
            nc.vector.memset(ones[:], 1.0)
            ones1 = cp.tile([1, 128], f32)
            nc.vector.memset(ones1[:], 1.0)

            # Wh in k-major 128x128 blocks, rounded to fp32r (scalar queue):
            # wh_r[:, (k*8+t)*128 + c] = Wh[128 t + p, 128 k + c]
            # k=0 blocks load first, then the first s-block of batch 0 is
            # prefetched on the same queue, then the remaining k blocks.
            wh_r = whrp.tile([128, NK * H], f32r)

            def load_wh_k(k):
                for t in range(NK):
                    wb = wstp.tile([128, 128], f32, tag="wst")
                    nc.sync.dma_start(
                        wb[:], wh_d[t * 128:(t + 1) * 128, k * 128:(k + 1) * 128])
                    nc.vector.tensor_copy(
                        wh_r[:, (k * NK + t) * 128:(k * NK + t + 1) * 128], wb[:])

            def load_xt(b, g, split=False):
                xt = xtp.tile([128, NK * SBW], f32r, tag="xt", name=f"xt_{b}_{g}")
                for t in range(NK):
                    xtr = xtrp.tile([128, SBW], f32, tag="xtr")
                    eng = nc.scalar if (split and t % 2) else nc.sync
                    eng.dma_start(
                        xtr[:], eo_t[b, t * 128:(t + 1) * 128,
                                     g * SBW:(g + 1) * SBW])
                    nc.vector.tensor_copy(xt[:, t * SBW:(t + 1) * SBW], xtr[:])
                return xt

            load_wh_k(0)
            xt_prefetch = {(0, 0): load_xt(0, 0)}
            for k in range(1, NK):
                load_wh_k(k)

            # V -> v_sb[p, t] = V[128 t + p]
            v8r = cp.tile([8, 128], f32r)
            nc.vector.tensor_copy(v8r[:], v8[:])
            ps_v = ps_s.tile([128, 8], f32r, tag="small")
            nc.tensor.transpose(ps_v[:], v8r[:], ident_r[0:8, 0:8])
            v_sb = cp.tile([128, 8], f32)
            nc.vector.tensor_copy(v_sb[:], ps_v[:])

            # (bh + bs) -> bhsT[p, t]
            bhs8 = cp.tile([8, 128], f32r)
            nc.vector.tensor_add(out=bhs8[:], in0=bh8[:], in1=bs8[:])
            ps_b = ps_s.tile([128, 8], f32r, tag="small")
            nc.tensor.transpose(ps_b[:], bhs8[:], ident_r[0:8, 0:8])
            bhsT = cp.tile([128, 8], f32)
            nc.vector.tensor_copy(bhsT[:], ps_b[:])

            # decoder state -> dT[p, 4t + b] = d[b, 128 t + p]  (fp32r)
            d8r = cp.tile([BL, H], f32r)
            nc.vector.tensor_copy(d8r[:], d8[:])
            dT = cp.tile([128, NK * BL], f32r)
            for t in range(NK):
                ps_d = ps_s.tile([128, BL], f32r, tag="small")
                nc.tensor.transpose(ps_d[:], d8r[0:BL, t * 128:(t + 1) * 128],
                                    ident_r[0:BL, 0:BL])
                nc.vector.tensor_copy(dT[:, t * BL:(t + 1) * BL], ps_d[:])

            # cov_p per batch (c, wc in (16,128) layout)
            cc = cp.tile([NJ, BL], f32)
            junk = cp.tile([NJ, 128], f32)
            for b in range(BL):
                nc.vector.tensor_mul(out=junk[:], in0=c_sbs[b][:], in1=wc_sb[:])
                nc.vector.reduce_sum(out=cc[0:NJ, b:b + 1], in_=junk[:],
                                     axis=mybir.AxisListType.X)
            ps_cp = ps_s.tile([1, BL], f32, tag="small")
            nc.tensor.matmul(ps_cp[:], ones[0:NJ, :], cc[:], start=True, stop=True)
            covp1 = cp.tile([1, BL], f32)
            nc.vector.tensor_scalar(out=covp1[:], in0=ps_cp[:], scalar1=bc_sb[:],
                                    scalar2=None, op0=ALU.add)
            ps_cb = ps_s.tile([128, BL], f32, tag="small")
            nc.tensor.matmul(ps_cb[:], ones1[:], covp1[:], start=True, stop=True)
            covp_bc = cp.tile([128, BL], f32)
            nc.vector.tensor_copy(covp_bc[:], ps_cb[:])

            # dec_p in natural layout: (4 b, 1024 k) = dT.T @ Ws, fp32r N=512.
            # Ws streams on the gpsimd (SWDGE) queue.
            ps_dn0 = ps_c.tile([BL, SBW], f32, tag="ctx")
            ps_dn1 = ps_s.tile([BL, SBW], f32, tag="small")
            for t in range(NK):
                wst = wraw.tile([128, H], f32, tag="wraw")
                nc.scalar.dma_start(wst[:], ws_d[t * 128:(t + 1) * 128, :])
                wsr = wraw.tile([128, H], f32r, tag="wsr")
                nc.vector.tensor_copy(wsr[:], wst[:])
                nc.tensor.matmul(ps_dn0[:], dT[:, t * BL:(t + 1) * BL],
                                 wsr[:, 0:SBW], start=(t == 0), stop=(t == NK - 1))
                nc.tensor.matmul(ps_dn1[:], dT[:, t * BL:(t + 1) * BL],
                                 wsr[:, SBW:H], start=(t == 0), stop=(t == NK - 1))
            dn_sb = cp.tile([BL, H], f32r)
            nc.vector.tensor_copy(dn_sb[0:BL, 0:SBW], ps_dn0[:])
            nc.vector.tensor_copy(dn_sb[0:BL, SBW:H], ps_dn1[:])

            # bias_sb[p, 4 k + b] = dec_p^T + (bh+bs)^T + cov_p
            bias_sb = cp.tile([128, NK * BL], f32)
            for k in range(NK):
                ps_dk = ps_s.tile([128, BL], f32r, tag="small")
                nc.tensor.transpose(ps_dk[:], dn_sb[0:BL, k * 128:(k + 1) * 128],
                                    ident_r[0:BL, 0:BL])
                nc.vector.tensor_scalar(
                    out=bias_sb[:, k * BL:(k + 1) * BL], in0=ps_dk[:],
                    scalar1=bhsT[:, k:k + 1], scalar2=None, op0=ALU.add)
                nc.vector.tensor_add(
                    out=bias_sb[:, k * BL:(k + 1) * BL],
                    in0=bias_sb[:, k * BL:(k + 1) * BL], in1=covp_bc[:])

            # ---------------- main per-batch pipeline ----------------
            for b in range(BL):
                U = up.tile([128, S], f32, tag="u")

                for g in range(NG):
                    xt = xt_prefetch.pop((b, g), None)
                    if xt is None:
                        xt = load_xt(b, g)

                    for k in range(NK):
                        ps_mm = ps_e.tile([128, SBW], f32, tag="mm")
                        for t in range(NK):
                            nc.tensor.matmul(
                                ps_mm[:],
                                wh_r[:, (k * NK + t) * 128:(k * NK + t + 1) * 128],
                                xt[:, t * SBW:(t + 1) * SBW],
                                start=(t == 0), stop=(t == NK - 1))
                        tt = tp.tile([128, SBW], f32, tag="t")
                        nc.scalar.activation(tt[:], ps_mm[:], AF.Tanh,
                                             bias=bias_sb[:, k * BL + b: k * BL + b + 1],
                                             scale=1.0)
                        if k == 0:
                            nc.vector.tensor_scalar(
                                out=U[:, g * SBW:(g + 1) * SBW], in0=tt[:],
                                scalar1=v_sb[:, k:k + 1], scalar2=None, op0=ALU.mult)
                        else:
                            nc.vector.affine_then_add(
                                out=U[:, g * SBW:(g + 1) * SBW], in0=tt[:],
                                in1=U[:, g * SBW:(g + 1) * SBW],
                                scale=v_sb[:, k:k + 1], bias=0.0)

                # natural X m-tiles for context, streamed just-in-time
                # (gpsimd SWDGE queue; consumed by the ctx matmuls below)
                xcs = []
                for m in range(NJ):
                    xc32 = xc32p.tile([128, H], f32, tag="xc32")
                    nc.scalar.dma_start(xc32[:], eo[b, m * 128:(m + 1) * 128, :])
                    xcr = xcrp.tile([128, H], f32r, tag="xcr",
                                    name=f"xcr_{b}_{m}")
                    if m % 2 == 0:
                        nc.vector.tensor_copy(xcr[:], xc32[:])
                    else:
                        nc.scalar.copy(xcr[:], xc32[:])
                    xcs.append(xcr)

                # score^T: score_sb[p, m] = score[s = 128 m + p]
                score_sb = smp.tile([128, NJ], f32, tag="score")
                for m in range(NJ):
                    ps_sc = ps_s.tile([128, 1], f32, tag="small")
                    nc.tensor.matmul(ps_sc[:], U[:, m * 128:(m + 1) * 128], ones[:],
                                     start=True, stop=True)
                    nc.scalar.copy(score_sb[:, m:m + 1], ps_sc[:])

                # softmax over all 2048 (no max-subtraction; |score| <= ~16)
                esb = smp.tile([128, NJ], f32, tag="esb")
                acc = smp.tile([128, 1], f32, tag="acc")
                nc.scalar.activation(esb[:], score_sb[:], AF.Exp, accum_out=acc[:])
                ps_tot = ps_s.tile([1, 1], f32, tag="small")
                nc.tensor.matmul(ps_tot[:], acc[:], ones[:], start=True, stop=True)
                tot1 = smp.tile([1, 1], f32, tag="tot")
                nc.scalar.copy(tot1[:], ps_tot[:])
                rcp1 = smp.tile([1, 1], f32, tag="rcp")
                nc.vector.reciprocal(rcp1[:], tot1[:])
                ps_rb = ps_s.tile([128, 1], f32, tag="small")
                nc.tensor.matmul(ps_rb[:], ones1[:], rcp1[:], start=True, stop=True)
                rcp_sb = smp.tile([128, 1], f32, tag="rcpb")
                nc.scalar.copy(rcp_sb[:], ps_rb[:])

                attn_f = smp.tile([128, NJ], f32, tag="attnf")
                nc.vector.tensor_scalar_mul(attn_f[:], esb[:], rcp_sb[:])
                attn_r = smp.tile([128, NJ], f32r, tag="attnr")
                nc.vector.tensor_scalar_mul(attn_r[:], esb[:], rcp_sb[:])

                ps_at = ps_s.tile([NJ, 128], f32, tag="small")
                nc.tensor.transpose(ps_at[:], attn_f[:], ident_f[:])
                attn16 = smp.tile([NJ, 128], f32, tag="attn16")
                nc.vector.tensor_copy(attn16[:], ps_at[:])
                covn16 = smp.tile([NJ, 128], f32, tag="covn16")
                nc.vector.tensor_add(out=covn16[:], in0=c_sbs[b][:], in1=attn16[:])

                nc.gpsimd.dma_start(
                    attn_o[b].rearrange("(m q) one -> m (q one)", q=128), attn16[:])
                nc.gpsimd.dma_start(
                    covn_o[b].rearrange("(m q) -> m q", q=128), covn16[:])

                # context: ctx[h] = sum_m sum_p attn[p, m] * X[128 m + p, h]
                # m-major so each streamed xcr tile's two reads are adjacent
                def emit_ctx(b=b, attn_r=attn_r, xcs=xcs):
                    ctx_sb = smp.tile([1, H], f32, tag="ctxsb",
                                      name=f"ctx_sb_{b}")
                    ps_cx0 = ps_c.tile([1, SBW], f32, tag="ctx",
                                       name=f"ps_cx0_{b}")
                    ps_cx1 = ps_c.tile([1, SBW], f32, tag="ctx",
                                       name=f"ps_cx1_{b}")
                    for m in range(NJ):
                        nc.tensor.matmul(
                            ps_cx0[:], attn_r[:, m:m + 1], xcs[m][:, 0:SBW],
                            start=(m == 0), stop=(m == NJ - 1))
                        nc.tensor.matmul(
                            ps_cx1[:], attn_r[:, m:m + 1], xcs[m][:, SBW:H],
                            start=(m == 0), stop=(m == NJ - 1))
                    nc.scalar.copy(ctx_sb[:, 0:SBW], ps_cx0[:])
                    nc.scalar.copy(ctx_sb[:, SBW:H], ps_cx1[:])
                    nc.gpsimd.dma_start(ctx_o[b:b + 1, :], ctx_sb[:])

                emit_ctx()

    nc.compile()
    return nc


def _get_nc():
    if "nc" not in _CACHE:
        _CACHE["nc"] = _build()
    return _CACHE["nc"]


def run(trace=False, **inputs):
    from concourse.bass_utils import run_bass_kernel_spmd

    nc = _get_nc()
    f32 = np.float32

    def c(x):
        return np.ascontiguousarray(np.asarray(x, dtype=f32))

    in_maps = []
    for i in range(NCORES):
        sl = slice(i * BL, (i + 1) * BL)
        eo_sl = c(inputs["encoder_output"][sl])
        in_maps.append({
            "eo": eo_sl,
            "eo_t": np.ascontiguousarray(eo_sl.transpose(0, 2, 1)),
            "dec": c(inputs["decoder_state"][sl]),
            "cov": c(inputs["coverage_vector"][sl]),
            "wh": c(inputs["Wh"]),
            "bh": c(inputs["bh"]),
            "ws": c(inputs["Ws"]),
            "bs": c(inputs["bs"]),
            "v": c(inputs["V"]),
            "wc": c(inputs["Wc"]),
            "bc": c(inputs["bc"]),
        })

    res = run_bass_kernel_spmd(nc, in_maps, list(range(NCORES)), trace=trace)

    ctx = np.concatenate([res.results[i]["ctx_o"] for i in range(NCORES)], axis=0)
    attn = np.concatenate([res.results[i]["attn_o"] for i in range(NCORES)], axis=0)
    covn = np.concatenate([res.results[i]["covn_o"] for i in range(NCORES)], axis=0)
    return (ctx, attn, covn), res


def kernel(**inputs):
    outs, _ = run(trace=False, **inputs)
    return outs


# revision 18
# speedup vs baseline: 1.1733x; 1.0625x over previous
"""Bahdanau attention with coverage — Trainium2 Bass kernel, 8-core data-parallel.

Math (per batch b):
  enc_p = X_b @ Wh + bh            X_b: (S=2048, H=1024)
  dec_p = d_b @ Ws + bs            (H,)
  cov_p = c_b @ Wc + bc            scalar
  score = tanh(enc_p + dec_p + cov_p) @ V + bv      (bv dropped: softmax shift-invariant)
  attn  = softmax(score)           over S
  cov_new = c_b + attn
  ctx   = attn^T @ X_b             (H,)

Strategy: batch-sharded over 8 cores (4 batches each). Compute enc_p in the
transposed layout enc_p^T (H on partitions, S on free) so the bias add + tanh
fuse into one ScalarE activation and the V-dot contracts over partitions.
The host stages X in both layouts (transposed for the enc_p matmul, natural
streamed just-in-time for the context matmul) so no on-chip transposes of X
are needed. All big matmuls run in fp32r (1 cycle/row at N=512, ~16x more
accurate than bf16). Wh is loaded in k-major 128x128 blocks so the first
matmul group only waits for 512KB of weights.

Sequence index mapping is plain s-major: column c of U <-> s = c; attn tile
(p, m) <-> s = 128 m + p. attn/coverage get one small PE transpose to a
(16, 128) layout so the output DMAs are 512B-contiguous.
"""

import numpy as np

B, S, H = 32, 2048, 1024
NCORES = 8
BL = B // NCORES          # batches per core
NK = H // 128             # 8 h/k tiles
NJ = 16                   # 128-row s-tiles per batch
NG = 4                    # s-blocks of 512 per batch
SBW = 512                 # s-block width

_CACHE = {}


def _build():
    import concourse.bacc as bacc
    import concourse.mybir as mybir
    import concourse.tile as tile
    from concourse.masks import make_identity

    f32 = mybir.dt.float32
    f32r = mybir.dt.float32r
    AF = mybir.ActivationFunctionType
    ALU = mybir.AluOpType

    nc = bacc.Bacc("TRN2", target_bir_lowering=False, debug=False,
                   num_devices=NCORES)

    eo = nc.dram_tensor("eo", (BL, S, H), f32, kind="ExternalInput").ap()
    eo_t = nc.dram_tensor("eo_t", (BL, H, S), f32, kind="ExternalInput").ap()
    dec = nc.dram_tensor("dec", (BL, H), f32, kind="ExternalInput").ap()
    cov = nc.dram_tensor("cov", (BL, S), f32, kind="ExternalInput").ap()
    wh_d = nc.dram_tensor("wh", (H, H), f32, kind="ExternalInput").ap()
    bh_d = nc.dram_tensor("bh", (H,), f32, kind="ExternalInput").ap()
    ws_d = nc.dram_tensor("ws", (H, H), f32, kind="ExternalInput").ap()
    bs_d = nc.dram_tensor("bs", (H,), f32, kind="ExternalInput").ap()
    v_d = nc.dram_tensor("v", (H, 1), f32, kind="ExternalInput").ap()
    wc_d = nc.dram_tensor("wc", (S, 1), f32, kind="ExternalInput").ap()
    bc_d = nc.dram_tensor("bc", (1,), f32, kind="ExternalInput").ap()

    ctx_o = nc.dram_tensor("ctx_o", (BL, H), f32, kind="ExternalOutput").ap()
    attn_o = nc.dram_tensor("attn_o", (BL, S, 1), f32, kind="ExternalOutput").ap()
    covn_o = nc.dram_tensor("covn_o", (BL, S), f32, kind="ExternalOutput").ap()

    with tile.TileContext(nc) as tc:
        with (
            tc.tile_pool(name="const", bufs=1) as cp,
            tc.tile_pool(name="wraw", bufs=2) as wraw,
            tc.tile_pool(name="wst", bufs=4) as wstp,
            tc.tile_pool(name="whr", bufs=1) as whrp,
            tc.tile_pool(name="xtr", bufs=4) as xtrp,
            tc.tile_pool(name="xt", bufs=2) as xtp,
            tc.tile_pool(name="xc32", bufs=3) as xc32p,
            tc.tile_pool(name="xcr", bufs=12) as xcrp,
            tc.tile_pool(name="tt", bufs=3) as tp,
            tc.tile_pool(name="u", bufs=2) as up,
            tc.tile_pool(name="sm", bufs=2) as smp,
            tc.tile_pool(name="ps_e", bufs=4, space="PSUM") as ps_e,
            tc.tile_pool(name="ps_s", bufs=2, space="PSUM") as ps_s,
            tc.tile_pool(name="ps_c", bufs=2, space="PSUM") as ps_c,
        ):
            # ------- small input DMAs first (sync queue)
            v8 = cp.tile([8, 128], f32)
            nc.sync.dma_start(v8[:], v_d.rearrange("(t p) one -> t (p one)", t=8))
            bh8 = cp.tile([8, 128], f32)
            nc.sync.dma_start(bh8[:], bh_d.rearrange("(t p) -> t p", t=8))
            bs8 = cp.tile([8, 128], f32)
            nc.sync.dma_start(bs8[:], bs_d.rearrange("(t p) -> t p", t=8))
            d8 = wraw.tile([BL, H], f32, tag="wraw")
            nc.sync.dma_start(d8[:], dec[:, :])
            wc_sb = cp.tile([NJ, 128], f32)
            nc.sync.dma_start(wc_sb[:], wc_d.rearrange("(m q) one -> m (q one)", q=128))
            bc_sb = cp.tile([1, 1], f32)
            nc.sync.dma_start(bc_sb[:], bc_d.rearrange("(one two) -> one two", one=1))
            c_sbs = []
            for b in range(BL):
                c_sb = smp.tile([NJ, 128], f32, tag="c", bufs=BL)
                nc.sync.dma_start(c_sb[:], cov[b].rearrange("(m q) -> m q", q=128))
                c_sbs.append(c_sb)

            # ---------------- constants ----------------
            ident_f = cp.tile([128, 128], f32)
            make_identity(nc, ident_f[:])
            ident_r = cp.tile([128, 128], f32r)
            nc.vector.tensor_copy(ident_r[:], ident_f[:])

            ones = cp.tile([128, 1], f32)
            nc.vector.memset(ones[:], 1.0)
            ones1 = cp.tile([1, 128], f32)
            nc.vector.memset(ones1[:], 1.0)

            # Wh in k-major 128x128 blocks, rounded to fp32r (scalar queue):
            # wh_r[:, (k*8+t)*128 + c] = Wh[128 t + p, 128 k + c]
            # k=0 blocks load first, then the first s-block of batch 0 is
            # prefetched on the same queue, then the remaining k blocks.
            wh_r = whrp.tile([128, NK * H], f32r)

            def load_wh_k(k):
                eng = nc.sync if k % 2 == 0 else nc.scalar
                for t in range(NK):
                    wb = wstp.tile([128, 128], f32, tag="wst")
                    eng.dma_start(
                        wb[:], wh_d[t * 128:(t + 1) * 128, k * 128:(k + 1) * 128])
                    nc.vector.tensor_copy(
                        wh_r[:, (k * NK + t) * 128:(k * NK + t + 1) * 128], wb[:])

            def load_xt(b, g, split=False):
                xt = xtp.tile([128, NK * SBW], f32r, tag="xt", name=f"xt_{b}_{g}")
                for t in range(NK):
                    xtr = xtrp.tile([128, SBW], f32, tag="xtr")
                    eng = nc.scalar if (split and t % 2) else nc.sync
                    eng.dma_start(
                        xtr[:], eo_t[b, t * 128:(t + 1) * 128,
                                     g * SBW:(g + 1) * SBW])
                    nc.vector.tensor_copy(xt[:, t * SBW:(t + 1) * SBW], xtr[:])
                return xt

            load_wh_k(0)
            xt_prefetch = {(0, 0): load_xt(0, 0)}
            for k in range(1, NK):
                load_wh_k(k)

            # V -> v_sb[p, t] = V[128 t + p]
            v8r = cp.tile([8, 128], f32r)
            nc.vector.tensor_copy(v8r[:], v8[:])
            ps_v = ps_s.tile([128, 8], f32r, tag="small")
            nc.tensor.transpose(ps_v[:], v8r[:], ident_r[0:8, 0:8])
            v_sb = cp.tile([128, 8], f32)
            nc.vector.tensor_copy(v_sb[:], ps_v[:])

            # (bh + bs) -> bhsT[p, t]
            bhs8 = cp.tile([8, 128], f32r)
            nc.vector.tensor_add(out=bhs8[:], in0=bh8[:], in1=bs8[:])
            ps_b = ps_s.tile([128, 8], f32r, tag="small")
            nc.tensor.transpose(ps_b[:], bhs8[:], ident_r[0:8, 0:8])
            bhsT = cp.tile([128, 8], f32)
            nc.vector.tensor_copy(bhsT[:], ps_b[:])

            # decoder state -> dT[p, 4t + b] = d[b, 128 t + p]  (fp32r)
            d8r = cp.tile([BL, H], f32r)
            nc.vector.tensor_copy(d8r[:], d8[:])
            dT = cp.tile([128, NK * BL], f32r)
            for t in range(NK):
                ps_d = ps_s.tile([128, BL], f32r, tag="small")
                nc.tensor.transpose(ps_d[:], d8r[0:BL, t * 128:(t + 1) * 128],
                                    ident_r[0:BL, 0:BL])
                nc.vector.tensor_copy(dT[:, t * BL:(t + 1) * BL], ps_d[:])

            # cov_p per batch (c, wc in (16,128) layout)
            cc = cp.tile([NJ, BL], f32)
            junk = cp.tile([NJ, 128], f32)
            for b in range(BL):
                nc.vector.tensor_mul(out=junk[:], in0=c_sbs[b][:], in1=wc_sb[:])
                nc.vector.reduce_sum(out=cc[0:NJ, b:b + 1], in_=junk[:],
                                     axis=mybir.AxisListType.X)
            ps_cp = ps_s.tile([1, BL], f32, tag="small")
            nc.tensor.matmul(ps_cp[:], ones[0:NJ, :], cc[:], start=True, stop=True)
            covp1 = cp.tile([1, BL], f32)
            nc.vector.tensor_scalar(out=covp1[:], in0=ps_cp[:], scalar1=bc_sb[:],
                                    scalar2=None, op0=ALU.add)
            ps_cb = ps_s.tile([128, BL], f32, tag="small")
            nc.tensor.matmul(ps_cb[:], ones1[:], covp1[:], start=True, stop=True)
            covp_bc = cp.tile([128, BL], f32)
            nc.vector.tensor_copy(covp_bc[:], ps_cb[:])

            # dec_p in natural layout: (4 b, 1024 k) = dT.T @ Ws, fp32r N=512.
            # Ws streams on the gpsimd (SWDGE) queue.
            ps_dn0 = ps_c.tile([BL, SBW], f32, tag="ctx")
            ps_dn1 = ps_s.tile([BL, SBW], f32, tag="small")
            for t in range(NK):
                wst = wraw.tile([128, H], f32, tag="wraw")
                nc.scalar.dma_start(wst[:], ws_d[t * 128:(t + 1) * 128, :])
                wsr = wraw.tile([128, H], f32r, tag="wsr")
                nc.vector.tensor_copy(wsr[:], wst[:])
                nc.tensor.matmul(ps_dn0[:], dT[:, t * BL:(t + 1) * BL],
                                 wsr[:, 0:SBW], start=(t == 0), stop=(t == NK - 1))
                nc.tensor.matmul(ps_dn1[:], dT[:, t * BL:(t + 1) * BL],
                                 wsr[:, SBW:H], start=(t == 0), stop=(t == NK - 1))
            dn_sb = cp.tile([BL, H], f32r)
            nc.vector.tensor_copy(dn_sb[0:BL, 0:SBW], ps_dn0[:])
            nc.vector.tensor_copy(dn_sb[0:BL, SBW:H], ps_dn1[:])

            # bias_sb[p, 4 k + b] = dec_p^T + (bh+bs)^T + cov_p
            bias_sb = cp.tile([128, NK * BL], f32)
            for k in range(NK):
                ps_dk = ps_s.tile([128, BL], f32r, tag="small")
                nc.tensor.transpose(ps_dk[:], dn_sb[0:BL, k * 128:(k + 1) * 128],
                                    ident_r[0:BL, 0:BL])
                nc.vector.tensor_scalar(
                    out=bias_sb[:, k * BL:(k + 1) * BL], in0=ps_dk[:],
                    scalar1=bhsT[:, k:k + 1], scalar2=None, op0=ALU.add)
                nc.vector.tensor_add(
                    out=bias_sb[:, k * BL:(k + 1) * BL],
                    in0=bias_sb[:, k * BL:(k + 1) * BL], in1=covp_bc[:])

            # ---------------- main per-batch pipeline ----------------
            for b in range(BL):
                U = up.tile([128, S], f32, tag="u")

                for g in range(NG):
                    xt = xt_prefetch.pop((b, g), None)
                    if xt is None:
                        xt = load_xt(b, g)

                    for k in range(NK):
                        ps_mm = ps_e.tile([128, SBW], f32, tag="mm")
                        for t in range(NK):
                            nc.tensor.matmul(
                                ps_mm[:],
                                wh_r[:, (k * NK + t) * 128:(k * NK + t + 1) * 128],
                                xt[:, t * SBW:(t + 1) * SBW],
                                start=(t == 0), stop=(t == NK - 1))
                        tt = tp.tile([128, SBW], f32, tag="t")
                        nc.scalar.activation(tt[:], ps_mm[:], AF.Tanh,
                                             bias=bias_sb[:, k * BL + b: k * BL + b + 1],
                                             scale=1.0)
                        if k == 0:
                            nc.vector.tensor_scalar(
                                out=U[:, g * SBW:(g + 1) * SBW], in0=tt[:],
                                scalar1=v_sb[:, k:k + 1], scalar2=None, op0=ALU.mult)
                        else:
                            nc.vector.affine_then_add(
                                out=U[:, g * SBW:(g + 1) * SBW], in0=tt[:],
                                in1=U[:, g * SBW:(g + 1) * SBW],
                                scale=v_sb[:, k:k + 1], bias=0.0)

                # natural X m-tiles for context, streamed just-in-time
                # (gpsimd SWDGE queue; consumed by the ctx matmuls below)
                xcs = []
                for m in range(NJ):
                    xc32 = xc32p.tile([128, H], f32, tag="xc32")
                    nc.scalar.dma_start(xc32[:], eo[b, m * 128:(m + 1) * 128, :])
                    xcr = xcrp.tile([128, H], f32r, tag="xcr",
                                    name=f"xcr_{b}_{m}")
                    if m % 2 == 0:
                        nc.vector.tensor_copy(xcr[:], xc32[:])
                    else:
                        nc.scalar.copy(xcr[:], xc32[:])
                    xcs.append(xcr)

                # score^T: score_sb[p, m] = score[s = 128 m + p]
                score_sb = smp.tile([128, NJ], f32, tag="score")
                for m in range(NJ):
                    ps_sc = ps_s.tile([128, 1], f32, tag="small")
                    nc.tensor.matmul(ps_sc[:], U[:, m * 128:(m + 1) * 128], ones[:],
                                     start=True, stop=True)
                    nc.scalar.copy(score_sb[:, m:m + 1], ps_sc[:])

                # softmax over all 2048 (no max-subtraction; |score| <= ~16)
                esb = smp.tile([128, NJ], f32, tag="esb")
                acc = smp.tile([128, 1], f32, tag="acc")
                nc.scalar.activation(esb[:], score_sb[:], AF.Exp, accum_out=acc[:])
                ps_tot = ps_s.tile([1, 1], f32, tag="small")
                nc.tensor.matmul(ps_tot[:], acc[:], ones[:], start=True, stop=True)
                tot1 = smp.tile([1, 1], f32, tag="tot")
                nc.scalar.copy(tot1[:], ps_tot[:])
                rcp1 = smp.tile([1, 1], f32, tag="rcp")
                nc.vector.reciprocal(rcp1[:], tot1[:])
                ps_rb = ps_s.tile([128, 1], f32, tag="small")
                nc.tensor.matmul(ps_rb[:], ones1[:], rcp1[:], start=True, stop=True)
                rcp_sb = smp.tile([128, 1], f32, tag="rcpb")
                nc.scalar.copy(rcp_sb[:], ps_rb[:])

                attn_f = smp.tile([128, NJ], f32, tag="attnf")
                nc.vector.tensor_scalar_mul(attn_f[:], esb[:], rcp_sb[:])
                attn_r = smp.tile([128, NJ], f32r, tag="attnr")
                nc.vector.tensor_scalar_mul(attn_r[:], esb[:], rcp_sb[:])

                ps_at = ps_s.tile([NJ, 128], f32, tag="small")
                nc.tensor.transpose(ps_at[:], attn_f[:], ident_f[:])
                attn16 = smp.tile([NJ, 128], f32, tag="attn16")
                nc.vector.tensor_copy(attn16[:], ps_at[:])
                covn16 = smp.tile([NJ, 128], f32, tag="covn16")
                nc.vector.tensor_add(out=covn16[:], in0=c_sbs[b][:], in1=attn16[:])

                nc.gpsimd.dma_start(
                    attn_o[b].rearrange("(m q) one -> m (q one)", q=128), attn16[:])
                nc.gpsimd.dma_start(
                    covn_o[b].rearrange("(m q) -> m q", q=128), covn16[:])

                # context: ctx[h] = sum_m sum_p attn[p, m] * X[128 m + p, h]
                # m-major so each streamed xcr tile's two reads are adjacent
                def emit_ctx(b=b, attn_r=attn_r, xcs=xcs):
                    ctx_sb = smp.tile([1, H], f32, tag="ctxsb",
                                      name=f"ctx_sb_{b}")
                    ps_cx0 = ps_c.tile([1, SBW], f32, tag="ctx",
                                       name=f"ps_cx0_{b}")
                    ps_cx1 = ps_c.tile([1, SBW], f32, tag="ctx",
                                       name=f"ps_cx1_{b}")
                    for m in range(NJ):
                        nc.tensor.matmul(
                            ps_cx0[:], attn_r[:, m:m + 1], xcs[m][:, 0:SBW],
                            start=(m == 0), stop=(m == NJ - 1))
                        nc.tensor.matmul(
                            ps_cx1[:], attn_r[:, m:m + 1], xcs[m][:, SBW:H],
                            start=(m == 0), stop=(m == NJ - 1))
                    nc.scalar.copy(ctx_sb[:, 0:SBW], ps_cx0[:])
                    nc.scalar.copy(ctx_sb[:, SBW:H], ps_cx1[:])
                    nc.gpsimd.dma_start(ctx_o[b:b + 1, :], ctx_sb[:])

                emit_ctx()

    nc.compile()
    return nc


def _get_nc():
    if "nc" not in _CACHE:
        _CACHE["nc"] = _build()
    return _CACHE["nc"]


def run(trace=False, **inputs):
    from concourse.bass_utils import run_bass_kernel_spmd

    nc = _get_nc()
    f32 = np.float32

    def c(x):
        return np.ascontiguousarray(np.asarray(x, dtype=f32))

    in_maps = []
    for i in range(NCORES):
        sl = slice(i * BL, (i + 1) * BL)
        eo_sl = c(inputs["encoder_output"][sl])
        in_maps.append({
            "eo": eo_sl,
            "eo_t": np.ascontiguousarray(eo_sl.transpose(0, 2, 1)),
            "dec": c(inputs["decoder_state"][sl]),
            "cov": c(inputs["coverage_vector"][sl]),
            "wh": c(inputs["Wh"]),
            "bh": c(inputs["bh"]),
            "ws": c(inputs["Ws"]),
            "bs": c(inputs["bs"]),
            "v": c(inputs["V"]),
            "wc": c(inputs["Wc"]),
            "bc": c(inputs["bc"]),
        })

    res = run_bass_kernel_spmd(nc, in_maps, list(range(NCORES)), trace=trace)

    ctx = np.concatenate([res.results[i]["ctx_o"] for i in range(NCORES)], axis=0)
    attn = np.concatenate([res.results[i]["attn_o"] for i in range(NCORES)], axis=0)
    covn = np.concatenate([res.results[i]["covn_o"] for i in range(NCORES)], axis=0)
    return (ctx, attn, covn), res


def kernel(**inputs):
    outs, _ = run(trace=False, **inputs)
    return outs
